# revision 2
# baseline (speedup 1.0000x reference)
"""DisplaceChannel (integer displace + per-position 5x5 gaussian depthwise
conv) as a Bass/Tile kernel for 8 Trainium2 NeuronCores.

Math: the 5x5 gaussian kernel is separable and its normalizer factorizes;
the integer shift + 'same' zero-padding fold into banded 64x64 row/col
operators built host-side from the tiny (48,2) `offset`.  Per image:

    out = R1^T @ X @ R2        (R1 = y-operator, R2 = x-operator)

Device scheme (per batch-pair bp and position-group g, 8 channels):

  pass1 (x4, one per channel pair m): stationary lhsT = image pair
      [128(y_A|y_B), 128(u,x)] (fp16, FWL-eligible 128-col weight load),
      moving rhs = blockdiag(R1,R1) [128, 128] -> one N=128 matmul gives
      ps1[(u,x), (s,y')] for both batches of the pair at once.
  cast: ps1 [128,512] fp32 -> l2 fp16 (vector engine)
  pass2: stationary lhsT = blockdiag(R2,R2) [128,128], moving rhs = l2
      [128, 512] -> one N=512 matmul: ps2[(u,x_out), (m,s,y')].
  cast: ps2 -> outs fp16 (scalar engine), DMA out every 6 groups.

vs the previous version (16 N=64 matmuls + 4 N=128 matmuls per group)
this runs 5 matmuls with 128-wide weight loads, cutting PE time ~2x so
compute stays under the HBM roofline (~28 MB of fp16 I/O per core).

DMA queues: input chunks stream on the sync (SP HWDGE) ring in consume
order; ops and output chunks ride the scalar (ACT HWDGE) ring, so the
two streams never head-block each other and gpsimd/SWDGE is unused.

Sharding: data-parallel over batch (4 per core); operators replicated.
Host packs input fp16 as [bp, 128(y_A|y_B), (c,x)] and unpacks the
output so every device DMA moves multi-KB contiguous runs per partition.
"""

import numpy as np

from concourse import bacc, mybir, tile
from concourse.bass_utils import run_bass_kernel_spmd

# problem constants (hardcoded per harness contract)
B_FULL, C, H, W = 32, 384, 64, 64
N_CORES = 8
B_LOC = B_FULL // N_CORES          # 4 batches per core
P_POS = 48                         # offset positions; C // P_POS = 8 chan/pos
GROUP = C // P_POS                 # 8 channels share one operator pair
KSZ, SIGMA, CK = 5, 0.5, 2

N_BPAIR = B_LOC // 2               # batch-pairs (2bp, 2bp+1) per core
OUT_GROUPS = 6                     # groups per output chunk (48 channels)
OUT_COLS = OUT_GROUPS * GROUP * 64  # 3072 cols per out-chunk
XCOLS = C * 64                     # 24576 per-bp packed cols
OPS_SPLIT = 8                      # groups in the early ops tiles

FP16 = mybir.dt.float16
FP32 = mybir.dt.float32

_LAST_RESULT = None                # test.py introspection (profile/exec time)


def _shift_conv_matrix(sub, d):
    """[64(src), 64(out)] with R[src,out] = k[i], src = out + i - 2 - d,
    masked by conv zero-pad (0<=out+i-2<64) and shift zero-fill (0<=src<64)."""
    k = np.exp(-((np.arange(KSZ) - CK + sub) ** 2) / (2.0 * SIGMA**2))
    k = k / k.sum()
    R = np.zeros((H, H), dtype=np.float64)
    out = np.arange(H)
    for i in range(KSZ):
        t = out + i - CK            # coordinate in the shifted image
        src = t - d
        m = (t >= 0) & (t < H) & (src >= 0) & (src < H)
        R[src[m], out[m]] += k[i]
    return R


def _build_ops(offset):
    """ops1/ops2 [128, 48*128] fp16: per position blockdiag(R, R) with
    R1 = y-operator (pass1 rhs), R2 = x-operator (pass2 lhsT)."""
    off_round = np.round(offset.astype(np.float64))
    off_int = off_round.astype(np.int64)
    sub = offset.astype(np.float64) - off_round
    ops1 = np.zeros((128, P_POS * 128), dtype=np.float64)
    ops2 = np.zeros((128, P_POS * 128), dtype=np.float64)
    for p in range(P_POS):
        R1 = _shift_conv_matrix(sub[p, 1], off_int[p, 1])   # y: suby, dy
        R2 = _shift_conv_matrix(sub[p, 0], off_int[p, 0])   # x: subx, dx
        ops1[0:64, 128 * p:128 * p + 64] = R1
        ops1[64:128, 128 * p + 64:128 * p + 128] = R1
        ops2[0:64, 128 * p:128 * p + 64] = R2
        ops2[64:128, 128 * p + 64:128 * p + 128] = R2
    return ops1.astype(np.float16), ops2.astype(np.float16)


def _build_bass():
    nc = bacc.Bacc(
        "TRN2",
        target_bir_lowering=False,
        debug=False,
        num_devices=N_CORES,
    )
    # packed fp16 input: per bp a [128, 24576] block; channel c at cols
    # 64c:64c+64, batch 2bp y-rows on partitions 0:64, batch 2bp+1 on 64:128.
    x_in = nc.declare_dram_parameter("x", [N_BPAIR, 128, XCOLS], FP16,
                                     isOutput=False)
    ops1_in = nc.declare_dram_parameter("ops1", [128, P_POS * 128], FP16,
                                        isOutput=False)
    ops2_in = nc.declare_dram_parameter("ops2", [128, P_POS * 128], FP16,
                                        isOutput=False)
    # packed output: per bp [128, 24576] fp16 (host upcasts to f32);
    # partitions (u, x_out), cols (g, m, s, y'); channel = 8g + 2m + u,
    # batch = 2bp + s.
    y_out = nc.declare_dram_parameter("y", [N_BPAIR, 128, XCOLS], FP16,
                                      isOutput=True)

    with tile.TileContext(nc) as tc:
        with (
            tc.tile_pool(name="consts", bufs=1) as consts,
            tc.tile_pool(name="wchunk", bufs=1) as wpool,
            tc.tile_pool(name="l2", bufs=6) as l2pool,
            tc.tile_pool(name="outs", bufs=3) as outpool,
            tc.tile_pool(name="psum1", bufs=4, space="PSUM") as psum1p,
            tc.tile_pool(name="psum2", bufs=3, space="PSUM") as psum2p,
        ):
            # ops split into early/late tiles so group 0's compute only
            # waits on the small early transfer (ACT HWDGE ring).
            nb = P_POS - OPS_SPLIT
            t_ops1a = consts.tile([128, OPS_SPLIT * 128], FP16, tag="o1a")
            t_ops2a = consts.tile([128, OPS_SPLIT * 128], FP16, tag="o2a")
            t_ops1b = consts.tile([128, nb * 128], FP16, tag="o1b")
            t_ops2b = consts.tile([128, nb * 128], FP16, tag="o2b")
            nc.scalar.dma_start(out=t_ops1a[:],
                                in_=ops1_in[:, 0:OPS_SPLIT * 128])
            nc.scalar.dma_start(out=t_ops2a[:],
                                in_=ops2_in[:, 0:OPS_SPLIT * 128])
            nc.scalar.dma_start(out=t_ops1b[:],
                                in_=ops1_in[:, OPS_SPLIT * 128:])
            nc.scalar.dma_start(out=t_ops2b[:],
                                in_=ops2_in[:, OPS_SPLIT * 128:])

            def ops_slice(t_a, t_b, g):
                if g < OPS_SPLIT:
                    return t_a[:, 128 * g:128 * g + 128]
                go = g - OPS_SPLIT
                return t_b[:, 128 * go:128 * go + 128]

            # input chunks, all emitted upfront on the sync (SP HWDGE)
            # ring in consume order; every chunk gets its own buffer so
            # the stream never stalls on SBUF reuse.
            chunk_list = []                     # (bp, ga, gb)
            for bp in range(N_BPAIR):
                starts = ([0, 2, 4, 8, 16, 28, 40] if bp == 0
                          else [0, 12, 24, 36])
                for ci, a in enumerate(starts):
                    b = starts[ci + 1] if ci + 1 < len(starts) else P_POS
                    chunk_list.append((bp, a, b))
            g2chunk = {}
            for ci, (bp, a, b) in enumerate(chunk_list):
                for g in range(a, b):
                    g2chunk[(bp, g)] = ci
            wts = {}
            for ci, (bp, a, b) in enumerate(chunk_list):
                wt = wpool.tile([128, (b - a) * GROUP * 64], FP16,
                                tag=f"w{ci}")
                nc.sync.dma_start(
                    out=wt[:],
                    in_=x_in[bp][:, a * GROUP * 64:b * GROUP * 64])
                wts[ci] = wt

            # main pipeline: pass2 for slot j is emitted L slots after
            # its pass1, so the PE never stalls on the fp32->fp16 cast.
            slots = [(bp, g) for bp in range(N_BPAIR) for g in range(P_POS)]
            L = 3
            state = {}
            outs_t = None
            for slot in range(len(slots) + L):
                if slot < len(slots):
                    bp, g = slots[slot]
                    ci = g2chunk[(bp, g)]
                    wt = wts[ci]
                    goff = g - chunk_list[ci][1]
                    rhs1 = ops_slice(t_ops1a, t_ops1b, g)
                    ps1 = psum1p.tile([128, 512], FP32)
                    for m in range(4):          # channel pairs
                        cs = 512 * goff + 128 * m
                        nc.tensor.matmul(ps1[:, 128 * m:128 * m + 128],
                                         wt[:, cs:cs + 128], rhs1,
                                         start=True, stop=True)
                    l2 = l2pool.tile([128, 512], FP16)
                    nc.vector.tensor_copy(l2[:], ps1[:])
                    state[slot] = (bp, g, l2)
                j = slot - L
                if j >= 0:
                    bpj, gj, l2j = state.pop(j)
                    lhs2 = ops_slice(t_ops2a, t_ops2b, gj)
                    ps2 = psum2p.tile([128, 512], FP32)
                    nc.tensor.matmul(ps2[:, :], lhs2, l2j[:],
                                     start=True, stop=True)
                    if gj % OUT_GROUPS == 0:
                        outs_t = outpool.tile([128, OUT_COLS], FP16)
                    od = outs_t[:, 512 * (gj % OUT_GROUPS):
                                512 * (gj % OUT_GROUPS) + 512]
                    nc.scalar.copy(od, ps2[:])
                    if gj % OUT_GROUPS == OUT_GROUPS - 1:
                        oc = gj // OUT_GROUPS
                        nc.scalar.dma_start(
                            out=y_out[bpj][:, OUT_COLS * oc:
                                           OUT_COLS * (oc + 1)],
                            in_=outs_t[:])
    nc.compile()
    return nc


_NC_CACHE = None


def kernel(x: np.ndarray, offset: np.ndarray) -> np.ndarray:
    global _LAST_RESULT, _NC_CACHE
    assert x.shape == (B_FULL, C, H, W), x.shape
    ops1, ops2 = _build_ops(np.asarray(offset, dtype=np.float32))
    if _NC_CACHE is None:
        _NC_CACHE = _build_bass()
    nc = _NC_CACHE

    # host pack: fp16 cast + [p, (c, x)] layout; batch 2bp y-rows on
    # partitions 0:64, batch 2bp+1 y-rows on 64:128 (index permutation only).
    x16 = np.asarray(x, dtype=np.float32).astype(np.float16)
    xv = x16.reshape(N_CORES, N_BPAIR, 2, C, H, W)
    xP = np.empty((N_CORES, N_BPAIR, 128, C, W), dtype=np.float16)
    xP[:, :, 0:64] = xv[:, :, 0].transpose(0, 1, 3, 2, 4)   # [i,bp,y,c,x]
    xP[:, :, 64:128] = xv[:, :, 1].transpose(0, 1, 3, 2, 4)
    xP = xP.reshape(N_CORES, N_BPAIR, 128, XCOLS)

    in_maps = []
    for i in range(N_CORES):
        in_maps.append({"x": xP[i], "ops1": ops1, "ops2": ops2})
    res = run_bass_kernel_spmd(nc, in_maps, list(range(N_CORES)))
    _LAST_RESULT = res

    # host unpack: y[i] [bp, (u, x_out), (g, m, s, y')];
    # channel = 8g + 2m + u, batch = 4i + 2bp + s.
    out = np.empty((B_FULL, C, H, W), dtype=np.float32)
    for i in range(N_CORES):
        yv = res.results[i]["y"].astype(np.float32).reshape(
            N_BPAIR, 2, W, P_POS, GROUP // 2, 2, H)
        yt = yv.transpose(0, 5, 3, 4, 1, 6, 2)   # bp s g m u y' x'
        out[4 * i:4 * i + 4] = yt.reshape(B_LOC, C, H, W)
    return out


if __name__ == "__main__":
    nc = _build_bass()
    print("bass program built ok")


# revision 5
# speedup vs baseline: 1.0315x; 1.0315x over previous
"""DisplaceChannel (integer displace + per-position 5x5 gaussian depthwise
conv) as a Bass/Tile kernel for 8 Trainium2 NeuronCores.

Math: the 5x5 gaussian kernel is separable and its normalizer factorizes;
the integer shift + 'same' zero-padding fold into banded 64x64 row/col
operators built host-side from the tiny (48,2) `offset`.  Per image:

    out = R1^T @ X @ R2        (R1 = y-operator, R2 = x-operator)

Device scheme (per batch-pair bp and position-group g, 8 channels):

  pass1 (x4, one per channel pair m): stationary lhsT = image pair
      [128(y_A|y_B), 128(u,x)] (fp16, FWL-eligible 128-col weight load),
      moving rhs = blockdiag(R1,R1) [128, 128] -> one N=128 matmul gives
      ps1[(u,x), (s,y')] for both batches of the pair at once.
  cast: ps1 [128,512] fp32 -> l2 fp16 (vector engine)
  pass2: stationary lhsT = blockdiag(R2,R2) [128,128], moving rhs = l2
      [128, 512] -> one N=512 matmul: ps2[(u,x_out), (m,s,y')].
  cast: ps2 -> outs fp16 (scalar engine), DMA out every 6 groups.

vs the previous version (16 N=64 matmuls + 4 N=128 matmuls per group)
this runs 5 matmuls with 128-wide weight loads, cutting PE time ~2x so
compute stays under the HBM roofline (~28 MB of fp16 I/O per core).

DMA queues: input chunks stream on the sync (SP HWDGE) ring in consume
order; ops and output chunks ride the scalar (ACT HWDGE) ring, so the
two streams never head-block each other and gpsimd/SWDGE is unused.

Sharding: data-parallel over batch (4 per core); operators replicated.
Host packs input fp16 as [bp, 128(y_A|y_B), (c,x)] and unpacks the
output so every device DMA moves multi-KB contiguous runs per partition.
"""

import numpy as np

from concourse import bacc, mybir, tile
from concourse.bass_utils import run_bass_kernel_spmd

# problem constants (hardcoded per harness contract)
B_FULL, C, H, W = 32, 384, 64, 64
N_CORES = 8
B_LOC = B_FULL // N_CORES          # 4 batches per core
P_POS = 48                         # offset positions; C // P_POS = 8 chan/pos
GROUP = C // P_POS                 # 8 channels share one operator pair
KSZ, SIGMA, CK = 5, 0.5, 2

N_BPAIR = B_LOC // 2               # batch-pairs (2bp, 2bp+1) per core
OUT_GROUPS = 6                     # groups per output chunk (48 channels)
OUT_COLS = OUT_GROUPS * GROUP * 64  # 3072 cols per out-chunk
XCOLS = C * 64                     # 24576 per-bp packed cols
OPS_SPLIT = 8                      # groups in the early ops tiles

FP16 = mybir.dt.float16
FP32 = mybir.dt.float32

_LAST_RESULT = None                # test.py introspection (profile/exec time)


def _shift_conv_matrix(sub, d):
    """[64(src), 64(out)] with R[src,out] = k[i], src = out + i - 2 - d,
    masked by conv zero-pad (0<=out+i-2<64) and shift zero-fill (0<=src<64)."""
    k = np.exp(-((np.arange(KSZ) - CK + sub) ** 2) / (2.0 * SIGMA**2))
    k = k / k.sum()
    R = np.zeros((H, H), dtype=np.float64)
    out = np.arange(H)
    for i in range(KSZ):
        t = out + i - CK            # coordinate in the shifted image
        src = t - d
        m = (t >= 0) & (t < H) & (src >= 0) & (src < H)
        R[src[m], out[m]] += k[i]
    return R


def _build_ops(offset):
    """ops1/ops2 [128, 48*128] fp16: per position blockdiag(R, R) with
    R1 = y-operator (pass1 rhs), R2 = x-operator (pass2 lhsT)."""
    off_round = np.round(offset.astype(np.float64))
    off_int = off_round.astype(np.int64)
    sub = offset.astype(np.float64) - off_round
    ops1 = np.zeros((128, P_POS * 128), dtype=np.float64)
    ops2 = np.zeros((128, P_POS * 128), dtype=np.float64)
    for p in range(P_POS):
        R1 = _shift_conv_matrix(sub[p, 1], off_int[p, 1])   # y: suby, dy
        R2 = _shift_conv_matrix(sub[p, 0], off_int[p, 0])   # x: subx, dx
        ops1[0:64, 128 * p:128 * p + 64] = R1
        ops1[64:128, 128 * p + 64:128 * p + 128] = R1
        ops2[0:64, 128 * p:128 * p + 64] = R2
        ops2[64:128, 128 * p + 64:128 * p + 128] = R2
    return ops1.astype(np.float16), ops2.astype(np.float16)


def _build_bass():
    nc = bacc.Bacc(
        "TRN2",
        target_bir_lowering=False,
        debug=False,
        num_devices=N_CORES,
    )
    # packed fp16 input: per bp a [128, 24576] block; channel c at cols
    # 64c:64c+64, batch 2bp y-rows on partitions 0:64, batch 2bp+1 on 64:128.
    x_in = nc.declare_dram_parameter("x", [N_BPAIR, 128, XCOLS], FP16,
                                     isOutput=False)
    ops1_in = nc.declare_dram_parameter("ops1", [128, P_POS * 128], FP16,
                                        isOutput=False)
    ops2_in = nc.declare_dram_parameter("ops2", [128, P_POS * 128], FP16,
                                        isOutput=False)
    # packed output: per bp [128, 24576] fp16 (host upcasts to f32);
    # partitions (u, x_out), cols (g, m, s, y'); channel = 8g + 2m + u,
    # batch = 2bp + s.
    y_out = nc.declare_dram_parameter("y", [N_BPAIR, 128, XCOLS], FP16,
                                      isOutput=True)

    with tile.TileContext(nc) as tc:
        with (
            tc.tile_pool(name="consts", bufs=1) as consts,
            tc.tile_pool(name="wchunk", bufs=1) as wpool,
            tc.tile_pool(name="l2", bufs=4) as l2pool,
            tc.tile_pool(name="outs", bufs=3) as outpool,
            tc.tile_pool(name="psum1", bufs=2, space="PSUM") as psum1p,
            tc.tile_pool(name="psum2", bufs=2, space="PSUM") as psum2p,
        ):
            # ops split into early/late tiles so group 0's compute only
            # waits on the small early transfer.  They ride the gpsimd
            # SWDGE ring, which neither data stream uses.
            nb = P_POS - OPS_SPLIT
            t_ops1a = consts.tile([128, OPS_SPLIT * 128], FP16, tag="o1a")
            t_ops2a = consts.tile([128, OPS_SPLIT * 128], FP16, tag="o2a")
            t_ops1b = consts.tile([128, nb * 128], FP16, tag="o1b")
            t_ops2b = consts.tile([128, nb * 128], FP16, tag="o2b")
            nc.gpsimd.dma_start(out=t_ops1a[:],
                                in_=ops1_in[:, 0:OPS_SPLIT * 128])
            nc.gpsimd.dma_start(out=t_ops2a[:],
                                in_=ops2_in[:, 0:OPS_SPLIT * 128])
            nc.gpsimd.dma_start(out=t_ops1b[:],
                                in_=ops1_in[:, OPS_SPLIT * 128:])
            nc.gpsimd.dma_start(out=t_ops2b[:],
                                in_=ops2_in[:, OPS_SPLIT * 128:])

            def ops_slice(t_a, t_b, g):
                if g < OPS_SPLIT:
                    return t_a[:, 128 * g:128 * g + 128]
                go = g - OPS_SPLIT
                return t_b[:, 128 * go:128 * go + 128]

            # input chunks, all emitted upfront on the sync (SP HWDGE)
            # ring in consume order; every chunk gets its own buffer so
            # the stream never stalls on SBUF reuse.
            chunk_list = []                     # (bp, ga, gb)
            for bp in range(N_BPAIR):
                starts = ([0, 2, 4, 8, 16, 28, 40] if bp == 0
                          else [0, 12, 24, 36])
                for ci, a in enumerate(starts):
                    b = starts[ci + 1] if ci + 1 < len(starts) else P_POS
                    chunk_list.append((bp, a, b))
            g2chunk = {}
            for ci, (bp, a, b) in enumerate(chunk_list):
                for g in range(a, b):
                    g2chunk[(bp, g)] = ci
            wts = {}
            for ci, (bp, a, b) in enumerate(chunk_list):
                wt = wpool.tile([128, (b - a) * GROUP * 64], FP16,
                                tag=f"w{ci}")
                nc.sync.dma_start(
                    out=wt[:],
                    in_=x_in[bp][:, a * GROUP * 64:b * GROUP * 64])
                wts[ci] = wt

            # main pipeline: pass2 for slot j is emitted L slots after
            # its pass1, so the PE never stalls on the fp32->fp16 cast.
            # Slots are fused in PAIRS: one [128,1024] (2-bank) PSUM tile
            # and one cast instruction per pair, halving the fixed
            # per-instruction overhead on the vector/scalar engines.
            slots = [(bp, g) for bp in range(N_BPAIR) for g in range(P_POS)]
            L = 4                           # pair-aligned lookahead
            l2pairs = {}
            ps1pair = ps2pair = outs_t = None
            for slot in range(len(slots) + L):
                if slot < len(slots):
                    bp, g = slots[slot]
                    ci = g2chunk[(bp, g)]
                    wt = wts[ci]
                    goff = g - chunk_list[ci][1]
                    rhs1 = ops_slice(t_ops1a, t_ops1b, g)
                    if slot % 2 == 0:
                        ps1pair = psum1p.tile([128, 1024], FP32)
                    h1 = 512 * (slot % 2)
                    for m in range(4):          # channel pairs
                        cs = 512 * goff + 128 * m
                        nc.tensor.matmul(
                            ps1pair[:, h1 + 128 * m:h1 + 128 * m + 128],
                            wt[:, cs:cs + 128], rhs1,
                            start=True, stop=True)
                    if slot % 2 == 1:
                        l2pair = l2pool.tile([128, 1024], FP16)
                        nc.vector.tensor_copy(l2pair[:], ps1pair[:])
                        l2pairs[slot // 2] = l2pair
                j = slot - L
                if j >= 0:
                    bpj, gj = slots[j]
                    l2pair = l2pairs[j // 2]
                    lhs2 = ops_slice(t_ops2a, t_ops2b, gj)
                    h2 = 512 * (j % 2)
                    if j % 2 == 0:
                        ps2pair = psum2p.tile([128, 1024], FP32)
                    nc.tensor.matmul(ps2pair[:, h2:h2 + 512], lhs2,
                                     l2pair[:, h2:h2 + 512],
                                     start=True, stop=True)
                    if j % 2 == 1:
                        del l2pairs[j // 2]
                        pc = (gj // 2) % (OUT_GROUPS // 2)
                        if pc == 0:
                            outs_t = outpool.tile([128, OUT_COLS], FP16)
                        nc.scalar.copy(
                            outs_t[:, 1024 * pc:1024 * pc + 1024],
                            ps2pair[:])
                        if gj % OUT_GROUPS == OUT_GROUPS - 1:
                            oc = gj // OUT_GROUPS
                            nc.sync.dma_start(
                                out=y_out[bpj][:, OUT_COLS * oc:
                                               OUT_COLS * (oc + 1)],
                                in_=outs_t[:])
    nc.compile()
    return nc


_NC_CACHE = None


def kernel(x: np.ndarray, offset: np.ndarray) -> np.ndarray:
    global _LAST_RESULT, _NC_CACHE
    assert x.shape == (B_FULL, C, H, W), x.shape
    ops1, ops2 = _build_ops(np.asarray(offset, dtype=np.float32))
    if _NC_CACHE is None:
        _NC_CACHE = _build_bass()
    nc = _NC_CACHE

    # host pack: fp16 cast + [p, (c, x)] layout; batch 2bp y-rows on
    # partitions 0:64, batch 2bp+1 y-rows on 64:128 (index permutation only).
    x16 = np.asarray(x, dtype=np.float32).astype(np.float16)
    xv = x16.reshape(N_CORES, N_BPAIR, 2, C, H, W)
    xP = np.empty((N_CORES, N_BPAIR, 128, C, W), dtype=np.float16)
    xP[:, :, 0:64] = xv[:, :, 0].transpose(0, 1, 3, 2, 4)   # [i,bp,y,c,x]
    xP[:, :, 64:128] = xv[:, :, 1].transpose(0, 1, 3, 2, 4)
    xP = xP.reshape(N_CORES, N_BPAIR, 128, XCOLS)

    in_maps = []
    for i in range(N_CORES):
        in_maps.append({"x": xP[i], "ops1": ops1, "ops2": ops2})
    res = run_bass_kernel_spmd(nc, in_maps, list(range(N_CORES)))
    _LAST_RESULT = res

    # host unpack: y[i] [bp, (u, x_out), (g, m, s, y')];
    # channel = 8g + 2m + u, batch = 4i + 2bp + s.
    out = np.empty((B_FULL, C, H, W), dtype=np.float32)
    for i in range(N_CORES):
        yv = res.results[i]["y"].astype(np.float32).reshape(
            N_BPAIR, 2, W, P_POS, GROUP // 2, 2, H)
        yt = yv.transpose(0, 5, 3, 4, 1, 6, 2)   # bp s g m u y' x'
        out[4 * i:4 * i + 4] = yt.reshape(B_LOC, C, H, W)
    return out


if __name__ == "__main__":
    nc = _build_bass()
    print("bass program built ok")
